# revision 20
# baseline (speedup 1.0000x reference)
"""CopyGenerator kernel for Trainium2 (Bass/Tile), batch-parallel over 8 cores.

Core c owns batch c end-to-end. Key trick vs the dense baseline: for every
vocab column v NOT hit by src, the output is an affine function of the raw
generation score,
    out_v = log((1-a)/Z * exp(s_v)) = s_v + ln((1-a)/Z) = s_v + lnc1,
so pass 2 is a single DVE add per block (no dense Ln / blend / one-hot
matmul / scatter). The <=128 distinct src columns are emitted separately as
    out_fix_u = ln(e_u + Z*cpn_u) + lnc1,   cpn = (a/(1-a))*cp,
a tiny [128 x 128] side output; the host overwrites those columns during
the unshard (it knows the indices).

Padded vocab layout: V' = 32768 = 64 chunks of 512, each holding 500 real
vocab columns + 12 zero-embedding pads. This makes every matmul a full
512-f32 PSUM bank, every DVE copy contiguous, and every emb/out DMA an
8KB-per-partition contiguous transfer (host pre-packs emb into per-block
layout and strips the pads from the output). Pads contribute exp(0)=1 to
the softmax sum, corrected exactly via Z -= 768.

PE p-state note: the Tensor engine only reaches 2.4 GHz after ~3us of
continuous execution; any stall drops it to 1.2 GHz. Hence: deep emb
prefetch (bufs=6), attention emitted early (after block 1), attention
biases folded into the DVE PSUM->SBUF copies instead of extra matmuls,
and x/copy-gate computed with one 512-col matmul + a DVE dot.

Device pipeline per core (vocab' in 2048-col blocks; first two half-width):
  pass 1: DMA emb block -> gen matmul (PE fp8 DoubleRowSwInterleave) ->
          sc = s (f16, DVE copy/32) + exp(s) on ACT into scratch with fused
          row-sum -> Z
  attention after block 1, then fixup payload (E, cp, sg, e_g, cpn)
  Z -> lnc1 -> out_fix (tiny) ; pass 2 blocks only wait on lnc1:
  pass 2: out_f16 = sc + lnc1 per 4096-col block (DVE) -> DMA out
Output f16, host strips pads, patches fix columns, upcasts to f32.
"""

import sys

sys.path.insert(0, "/opt/trn_rl_repo")

import numpy as np

from concourse import bass, bacc, mybir
import concourse.tile as tile
from concourse.bass_utils import run_bass_kernel_spmd

NT, NS, BS, D, V = 128, 128, 8, 512, 32000
NCORES = 8
P = 128
KC = D // P  # 4 contraction chunks of 128
G = KC // 2  # 2 DoubleRow pair-groups (256-deep each)
RCH = 500  # real vocab cols per 512-chunk
CHP = 512  # padded chunk = one full PSUM bank
NCHK = V // RCH  # 64 chunks
VP = NCHK * CHP  # 32768 padded vocab
NPAD = NCHK * (CHP - RCH)  # 768 pad columns, each contributing exp(0)=1
WCH = 2 * CHP  # 1024: cols per exp activate / sc copy (2 banks)
DCH = 2 * WCH  # 2048: cols per emb DMA block
NDMA = VP // DCH  # 16
NZ = VP // WCH  # 32 partial-Z columns
# fine-grained half blocks at start (smooth 3-queue startup) and end
# (short Z-critical tail); 2048-col blocks in the middle
BLOCKS = (
    [(k * WCH, WCH) for k in range(6)]
    + [(6 * WCH + k * DCH, DCH) for k in range(12)]
    + [(VP - 2 * WCH, WCH), (VP - WCH, WCH)]
)
assert sum(w for _, w in BLOCKS) == VP
NPQ = 128  # fixup payload columns (unique src values, -1 padded)
ESCALE = 32.0  # host scales emb by 32 into fp8e4 normal range; exp undoes
F32 = mybir.dt.float32
F16 = mybir.dt.float16
F8 = mybir.dt.float8e4
AF = mybir.ActivationFunctionType
ALU = mybir.AluOpType
DR = mybir.MatmulPerfMode.DoubleRowSwInterleave
INV_SQRT_D = 1.0 / float(np.sqrt(np.float32(D)))


def build_kernel():
    nc = bacc.Bacc(
        "TRN2",
        target_bir_lowering=False,
        debug=False,
        enable_asserts=False,
        num_devices=NCORES,
    )
    emb8b = nc.dram_tensor("emb8b", [P, 2 * G * VP], F8, kind="ExternalInput").ap()
    hh8 = nc.dram_tensor("hh8", [P, G, 2 * NT], F8, kind="ExternalInput").ap()
    hhT = nc.dram_tensor("hhT", [P, KC, 2, P], F16, kind="ExternalInput").ap()
    qwT = nc.dram_tensor("qwT", [P, KC, D], F16, kind="ExternalInput").ap()
    qbT = nc.dram_tensor("qbT", [P, KC], F32, kind="ExternalInput").ap()
    qbbc = nc.dram_tensor("qbbc", [P, D], F16, kind="ExternalInput").ap()
    w2bc = nc.dram_tensor("w2bc", [P, D], F32, kind="ExternalInput").ap()
    b2bc = nc.dram_tensor("b2bc", [P, 1], F32, kind="ExternalInput").ap()
    iden = nc.dram_tensor("iden", [P, P], F32, kind="ExternalInput").ap()
    src = nc.dram_tensor("src", [NS, 1], F32, kind="ExternalInput").ap()
    valr = nc.dram_tensor("valr", [P, NPQ], F32, kind="ExternalInput").ap()
    embg8 = nc.dram_tensor("embg8", [P, G, 2 * NPQ], F8, kind="ExternalInput").ap()
    out = nc.dram_tensor("out", [NT, VP], F16, kind="ExternalOutput").ap()
    out_fix = nc.dram_tensor("out_fix", [NT, NPQ], F16, kind="ExternalOutput").ap()

    with tile.TileContext(nc) as tc:
        _emit(
            nc, tc, emb8b, hh8, hhT, qwT, qbT, qbbc, w2bc, b2bc, iden, src, valr,
            embg8, out, out_fix,
        )
    nc.compile()
    return nc


def _emit(
    nc, tc, emb8b, hh8, hhT, qwT, qbT, qbbc, w2bc, b2bc, iden, src, valr,
    embg8, out, out_fix,
):
    with (
        tc.tile_pool(name="persist", bufs=1) as pw,
        tc.tile_pool(name="small", bufs=2) as psm,
        tc.tile_pool(name="scr", bufs=3) as pscr,
        tc.tile_pool(name="ps_m", bufs=2, space="PSUM") as ps_m,
        tc.tile_pool(name="ps_fix", bufs=1, space="PSUM") as ps_fix,
        tc.tile_pool(name="ps_gen", bufs=2, space="PSUM") as ps_gen,
    ):
        # ---- persistent SBUF ----
        sc_sb = pw.tile([P, VP], F16)  # (t, v') raw gen scores - 64KB/part
        hh_sb = pw.tile([P, KC, 2, P], F16)  # (d, kc, {tgt,src}, t/s)
        hh8_sb = pw.tile([P, G, 2 * NT], F8)  # (d, g, swi(t)) DR weights
        qwT_sb = pw.tile([P, KC, D], F16)  # (d, kc, i)
        qbT_sb = pw.tile([P, KC], F32)  # (i % P, ic) q bias column
        qbbc_sb = pw.tile([P, D], F16)  # q bias row, tiled over partitions
        w2bc_sb = pw.tile([P, D], F32)  # fused copy-gate weight row, tiled
        b2bc_sb = pw.tile([P, 1], F32)
        qkT_sb = pw.tile([P, KC, 2, P], F16)  # (i, ic, {q,k}, t/s)
        k_sb = pw.tile([P, D], F16)  # (s, i)
        xw_sb = pw.tile([P, D], F32)  # x * w2 elementwise
        attn_sb = pw.tile([P, NS], F32)  # (t, s)
        attnT_sb = pw.tile([P, NT], F16)  # (s, t)
        a_sb = pw.tile([P, 1], F32)  # (t,)
        src_sb = pw.tile([P, 1], F32)  # (s,)
        valr_sb = pw.tile([P, NPQ], F32)
        embg8_sb = pw.tile([P, G, 2 * NPQ], F8)
        E_sb = pw.tile([P, NPQ], F16)  # (s, q) src==valr selection
        eg_sb = pw.tile([P, NPQ], F32)  # (t, q) exp(sg)
        cpn_sb = pw.tile([P, NPQ], F32)  # (t, q) (a/(1-a)) * cp
        tmp_sb = pw.tile([P, NPQ], F32)
        lntmp_sb = pw.tile([P, NPQ], F32)
        fix_sb = pw.tile([P, NPQ], F16)
        identity = pw.tile([P, P], F32)
        zparts = pw.tile([P, NZ], F32)
        zcol = pw.tile([P, 1], F32)
        oma_sb = pw.tile([P, 1], F32)
        aroma_sb = pw.tile([P, 1], F32)
        lnc1_sb = pw.tile([P, 1], F32)

        # preload ACT function tables (Exp/Ln/Sigmoid) so the lazy
        # ACT_TABLE_LOADs don't land mid-stream or in the Z->lnc1 chain
        dum = psm.tile([1, 1], F32, tag="dum")
        dum2 = psm.tile([1, 1], F32, tag="dum2")
        nc.vector.memset(dum[:], 1.0)
        nc.scalar.activation(out=dum2[:], in_=dum[:], func=AF.Exp)
        nc.scalar.activation(out=dum2[:], in_=dum[:], func=AF.Ln)
        nc.scalar.activation(out=dum2[:], in_=dum[:], func=AF.Sigmoid)

        # gen matmul dep goes first on the sync queue so the PE can start
        # asap; attention deps ride the idle gpsimd queue; emb blocks 0-5
        # alternate sync/scalar, later ones round-robin all three
        nc.sync.dma_start(out=hh8_sb[:], in_=hh8)

        def emit_smalls_a():
            nc.gpsimd.dma_start(out=hh_sb[:], in_=hhT)
            nc.gpsimd.dma_start(out=qwT_sb[:], in_=qwT)
            nc.gpsimd.dma_start(out=qbT_sb[:], in_=qbT)
            nc.gpsimd.dma_start(out=qbbc_sb[:], in_=qbbc)
            nc.gpsimd.dma_start(out=identity[:], in_=iden)

        def emit_smalls_b():
            nc.gpsimd.dma_start(out=src_sb[:], in_=src)
            nc.gpsimd.dma_start(out=valr_sb[:], in_=valr)
            nc.gpsimd.dma_start(out=embg8_sb[:], in_=embg8)
            nc.gpsimd.dma_start(out=w2bc_sb[:], in_=w2bc)
            nc.gpsimd.dma_start(out=b2bc_sb[:], in_=b2bc)

        def emit_attention():
            # q,k = h @ q_w.T + qb for {htgt, hsrc}; bias folded into the
            # PSUM->SBUF copy as a per-partition (i) scalar add
            for ic in range(KC):
                qkT_ps = ps_m.tile([P, 2 * P], F32, tag="m")
                for kc in range(KC):
                    nc.tensor.matmul(
                        out=qkT_ps[:],
                        lhsT=qwT_sb[:, kc, ic * P : (ic + 1) * P],
                        rhs=hh_sb[:, kc],
                        start=(kc == 0),
                        stop=(kc == KC - 1),
                    )
                nc.vector.tensor_scalar(
                    out=qkT_sb[:, ic],
                    in0=qkT_ps[:].rearrange("i (w t) -> i w t", t=P),
                    scalar1=qbT_sb[:, ic : ic + 1],
                    scalar2=None,
                    op0=ALU.add,
                )

            # k in (s, i) layout for the x matmul; bias is a row -> DVE add
            k_ps = ps_m.tile([P, D], F32, tag="m")
            for kc in range(KC):
                nc.tensor.matmul(
                    out=k_ps[:],
                    lhsT=hh_sb[:, kc, 1, :],
                    rhs=qwT_sb[:, kc, :],
                    start=(kc == 0),
                    stop=(kc == KC - 1),
                )
            nc.vector.tensor_tensor(
                out=k_sb[:], in0=k_ps[:], in1=qbbc_sb[:], op=ALU.add
            )

            s_ps = ps_m.tile([P, P], F32, tag="m")
            for ic in range(KC):
                nc.tensor.matmul(
                    out=s_ps[:],
                    lhsT=qkT_sb[:, ic, 0, :],
                    rhs=qkT_sb[:, ic, 1, :],
                    start=(ic == 0),
                    stop=(ic == KC - 1),
                )
            m_col = psm.tile([P, 1], F32, tag="m")
            negm = psm.tile([P, 1], F32, tag="negm")
            zatt = psm.tile([P, 1], F32, tag="zatt")
            rz = psm.tile([P, 1], F32, tag="rz")
            nc.vector.reduce_max(
                out=m_col[:], in_=s_ps[:], axis=mybir.AxisListType.X
            )
            nc.vector.tensor_scalar_mul(negm[:], m_col[:], -INV_SQRT_D)
            nc.scalar.activation(
                out=attn_sb[:],
                in_=s_ps[:],
                func=AF.Exp,
                bias=negm[:],
                scale=INV_SQRT_D,
                accum_out=zatt[:],
            )
            nc.vector.reciprocal(rz[:], zatt[:])
            nc.vector.tensor_scalar_mul(attn_sb[:], attn_sb[:], rz[:])

            t_ps = ps_m.tile([P, P], F32, tag="m")
            nc.tensor.transpose(t_ps[:], attn_sb[:], identity[:])
            nc.vector.tensor_copy(out=attnT_sb[:], in_=t_ps[:])

            # x = attn @ v (v == k): one 512-col matmul into (t, i) layout;
            # copy gate a = sigmoid(x . w2 + b2) via DVE dot
            x_ps = ps_m.tile([P, D], F32, tag="m")
            nc.tensor.matmul(
                out=x_ps[:],
                lhsT=attnT_sb[:],
                rhs=k_sb[:],
                start=True,
                stop=True,
            )
            c_col = psm.tile([P, 1], F32, tag="ccol")
            nc.vector.tensor_tensor(
                out=xw_sb[:], in0=x_ps[:], in1=w2bc_sb[:], op=ALU.mult
            )
            nc.vector.reduce_sum(
                out=c_col[:], in_=xw_sb[:], axis=mybir.AxisListType.X
            )
            nc.scalar.activation(
                out=a_sb[:], in_=c_col[:], func=AF.Sigmoid, bias=b2bc_sb[:]
            )

        def emit_fixup_payload():
            # oma = 1-a, aroma = a/(1-a)  (only needs a; Z comes later)
            roma = psm.tile([P, 1], F32, tag="roma")
            nc.vector.tensor_scalar(
                out=oma_sb[:],
                in0=a_sb[:],
                scalar1=-1.0,
                scalar2=1.0,
                op0=ALU.mult,
                op1=ALU.add,
            )
            nc.vector.reciprocal(roma[:], oma_sb[:])
            nc.vector.tensor_tensor(
                out=aroma_sb[:], in0=a_sb[:], in1=roma[:], op=ALU.mult
            )
            # E[s, q] = (src[s] == valr[q]); dead cols (valr=-1) never match
            nc.vector.tensor_scalar(
                out=E_sb[:],
                in0=valr_sb[:],
                scalar1=src_sb[:],
                scalar2=None,
                op0=ALU.is_equal,
            )
            # cp[t, q] = sum_s attn[t, s] * E[s, q];  cpn = aroma * cp
            cpu_ps = ps_fix.tile([P, CHP], F32, tag="cpu")
            nc.tensor.matmul(
                out=cpu_ps[:, 0:NPQ],
                lhsT=attnT_sb[:],
                rhs=E_sb[:],
                start=True,
                stop=True,
            )
            nc.vector.tensor_scalar(
                out=cpn_sb[:],
                in0=cpu_ps[:, 0:NPQ],
                scalar1=aroma_sb[:],
                scalar2=None,
                op0=ALU.mult,
            )
            # sg[t, q] = ESCALE * (htgt[t] . emb[valr[q]])  (same fp8 inputs
            # as the gen matmul -> identical rounding); e_g = exp(sg/ESCALE)
            sg_ps = ps_fix.tile([P, CHP], F32, tag="sg")
            for g in range(G):
                nc.tensor.matmul(
                    out=sg_ps[:, 0:NPQ],
                    lhsT=hh8_sb[:, g],
                    rhs=embg8_sb[:, g].rearrange("p (i q) -> p i q", i=2),
                    perf_mode=DR,
                    start=(g == 0),
                    stop=(g == G - 1),
                )
            nc.scalar.activation(
                out=eg_sb[:], in_=sg_ps[:, 0:NPQ], func=AF.Exp, scale=1.0 / ESCALE
            )

        # ---- pass 1: s = htgt @ embT, fp8 DoubleRow, streamed ----
        # first two loads are half-width so the PE starts sooner
        blocks = BLOCKS
        engs3 = [nc.sync, nc.scalar, nc.gpsimd]
        off = 0
        with (
            tc.tile_pool(name="embh", bufs=6) as pembh,
            tc.tile_pool(name="embst", bufs=8) as pemb,
        ):
            for bi, (v0, width) in enumerate(blocks):
                if width < DCH:
                    emb_t = pembh.tile([P, G, 2, width], F8, tag="embh")
                else:
                    emb_t = pemb.tile([P, G, 2, width], F8, tag="emb")
                eng = engs3[bi % 3]
                eng.dma_start(
                    out=emb_t[:].rearrange("p g i v -> p (g i v)"),
                    in_=emb8b[:, off : off + 2 * G * width],
                )
                off += 2 * G * width
                if bi == 2:
                    emit_smalls_a()
                    emit_smalls_b()
                for w in range(width // WCH):
                    # g-major: the DoubleRow weight set stays loaded for two
                    # consecutive matmuls (both PSUM banks of this group)
                    g_ps = ps_gen.tile([P, 2, CHP], F32, tag="g")
                    for g in range(G):
                        for h in range(2):
                            c0 = w * WCH + h * CHP
                            nc.tensor.matmul(
                                out=g_ps[:, h, :],
                                lhsT=hh8_sb[:, g],
                                rhs=emb_t[:, g, :, c0 : c0 + CHP],
                                perf_mode=DR,
                                start=(g == 0),
                                stop=(g == G - 1),
                            )
                    nz = (v0 + w * WCH) // WCH
                    scr = pscr.tile([P, 2, CHP], F16, tag="scr")
                    nc.scalar.activation(
                        out=scr[:],
                        in_=g_ps[:],
                        func=AF.Exp,
                        scale=1.0 / ESCALE,
                        accum_out=zparts[:, nz : nz + 1],
                    )
                    nc.vector.tensor_scalar(
                        out=sc_sb[:, nz * WCH : (nz + 1) * WCH].rearrange(
                            "t (h c) -> t h c", c=CHP
                        ),
                        in0=g_ps[:],
                        scalar1=1.0 / ESCALE,
                        scalar2=None,
                        op0=ALU.mult,
                    )
                if bi == 6:
                    emit_attention()
                    emit_fixup_payload()

        # ---- Z (pad-corrected), lnc1 = ln((1-a)/Z) ----
        zraw = psm.tile([P, 1], F32, tag="zraw")
        rzg = psm.tile([P, 1], F32, tag="rzg")
        c1t = psm.tile([P, 1], F32, tag="c1t")
        nc.vector.reduce_sum(out=zraw[:], in_=zparts[:], axis=mybir.AxisListType.X)
        nc.vector.tensor_scalar(
            out=zcol[:], in0=zraw[:], scalar1=-float(NPAD), scalar2=None, op0=ALU.add
        )
        nc.vector.reciprocal(rzg[:], zcol[:])
        nc.vector.tensor_tensor(out=c1t[:], in0=oma_sb[:], in1=rzg[:], op=ALU.mult)
        nc.scalar.activation(out=lnc1_sb[:], in_=c1t[:], func=AF.Ln)

        def emit_fix_chain():
            # out_fix = ln(e_g + Z*cpn) + lnc1 (off the pass-2 critical path)
            nc.vector.scalar_tensor_tensor(
                out=tmp_sb[:],
                in0=cpn_sb[:],
                scalar=zcol[:],
                in1=eg_sb[:],
                op0=ALU.mult,
                op1=ALU.add,
            )
            nc.scalar.activation(out=lntmp_sb[:], in_=tmp_sb[:], func=AF.Ln)
            nc.vector.tensor_scalar(
                out=fix_sb[:],
                in0=lntmp_sb[:],
                scalar1=lnc1_sb[:],
                scalar2=None,
                op0=ALU.add,
            )
            nc.gpsimd.dma_start(out=out_fix, in_=fix_sb[:])

        # ---- pass 2: out = sc + lnc1, f16, stream out ----
        OCH = 2 * DCH
        oblocks = (
            [(0, DCH)]
            + [(DCH + no * OCH, OCH) for no in range(7)]
            + [(VP - DCH, DCH)]
        )
        with tc.tile_pool(name="p2", bufs=2) as p2:
            for oi, (o0, owidth) in enumerate(oblocks):
                outt = p2.tile([P, OCH], F16, tag="outt")
                nc.vector.tensor_scalar(
                    out=outt[:, 0:owidth],
                    in0=sc_sb[:, o0 : o0 + owidth],
                    scalar1=lnc1_sb[:],
                    scalar2=None,
                    op0=ALU.add,
                )
                engs3[oi % 3].dma_start(
                    out=out[:, o0 : o0 + owidth], in_=outt[:, 0:owidth]
                )
                if oi == 1:
                    emit_fix_chain()


_NC_CACHE = []


def _get_nc():
    if not _NC_CACHE:
        _NC_CACHE.append(build_kernel())
    return _NC_CACHE[0]


def _f8(x):
    return x.astype(mybir.dt.np(F8))


def _make_in_maps(inputs):
    htgt = np.asarray(inputs["htgt"], dtype=np.float32)
    hsrc = np.asarray(inputs["hsrc"], dtype=np.float32)
    src = np.asarray(inputs["src"])
    srcf = src.astype(np.float32)  # exact for v < 2^24
    emb = np.asarray(inputs["emb_weight"], dtype=np.float32)
    q_w = np.asarray(inputs["q_w"], dtype=np.float32)
    q_b = np.asarray(inputs["q_b"], dtype=np.float32)
    f_w = np.asarray(inputs["f_w"], dtype=np.float32)
    f_b = np.asarray(inputs["f_b"], dtype=np.float32)
    copy_w = np.asarray(inputs["copy_w"], dtype=np.float32)
    copy_b = np.asarray(inputs["copy_b"], dtype=np.float32)

    # (G, 2, P, V): d = g*256 + i*128 + p, scaled into fp8e4 normal range
    emb8 = np.ascontiguousarray(_f8((emb.T * ESCALE).reshape(G, 2, P, V)))
    # padded layout: 64 chunks of 512 = 500 real + 12 zero cols
    embp = np.zeros((G, 2, P, NCHK, CHP), dtype=emb8.dtype)
    embp[:, :, :, :, 0:RCH] = emb8.reshape(G, 2, P, NCHK, RCH)
    embp = embp.reshape(G, 2, P, VP)
    # pack per (block, g, i, v') so each block is one contiguous
    # 8KB-per-partition DMA read
    segs = [
        np.transpose(embp[:, :, :, v0 : v0 + w], (2, 0, 1, 3)).reshape(P, -1)
        for (v0, w) in BLOCKS
    ]
    emb8b = np.ascontiguousarray(np.concatenate(segs, axis=1))

    qwT = np.ascontiguousarray(
        np.transpose(q_w.T.astype(np.float16).reshape(KC, P, D), (1, 0, 2))
    )
    qb16 = q_b.astype(np.float16)
    qbT = np.ascontiguousarray(q_b.astype(np.float32).reshape(KC, P).T)
    qbbc = np.ascontiguousarray(np.tile(qb16.reshape(1, D), (P, 1)))
    w2c = (f_w.T @ copy_w[0]).astype(np.float32)
    w2bc = np.ascontiguousarray(np.tile(w2c.reshape(1, D), (P, 1)))
    b2v = np.float32(copy_w[0] @ f_b + copy_b[0])
    b2bc = np.full((P, 1), b2v, np.float32)
    iden = np.ascontiguousarray(np.eye(P, dtype=np.float32))

    in_maps = []
    uniq_vals = []
    for c in range(NCORES):
        hh = np.stack([htgt[:, c, :].T, hsrc[:, c, :].T], axis=1)  # (D, 2, P)
        hhT = np.ascontiguousarray(
            np.transpose(hh.astype(np.float16).reshape(KC, P, 2, P), (1, 0, 2, 3))
        )
        # SwInterleave weight layout: per (g, partition): A/B pairs
        # interleaved per column, columns reversed
        h8 = _f8(htgt[:, c, :].T.reshape(G, 2, P, NT))
        rev = h8[:, :, :, ::-1]
        swi = np.empty((P, G, 2 * NT), dtype=h8.dtype)
        swi[:, :, 0::2] = np.transpose(rev[:, 0], (1, 0, 2))
        swi[:, :, 1::2] = np.transpose(rev[:, 1], (1, 0, 2))
        hh8 = np.ascontiguousarray(swi)

        # fixup prep: unique vocab values hit by this batch's src
        vals = np.unique(src[:, c].astype(np.int64))  # sorted, <= 128
        nu = len(vals)
        uniq_vals.append(vals)
        vq = np.full(NPQ, -1, np.int64)
        vq[:nu] = vals
        valr = np.ascontiguousarray(
            np.tile(vq.astype(np.float32).reshape(1, NPQ), (P, 1))
        )
        eg = np.zeros((G, 2, P, NPQ), dtype=emb8.dtype)
        eg[:, :, :, :nu] = emb8[:, :, :, vals]
        embg8 = np.ascontiguousarray(
            np.transpose(eg, (2, 0, 1, 3)).reshape(P, G, 2 * NPQ)
        )

        in_maps.append(
            {
                "emb8b": emb8b,
                "hh8": hh8,
                "hhT": hhT,
                "qwT": qwT,
                "qbT": qbT,
                "qbbc": qbbc,
                "w2bc": w2bc,
                "b2bc": b2bc,
                "iden": iden,
                "src": np.ascontiguousarray(srcf[:, c].reshape(NS, 1)),
                "valr": valr,
                "embg8": embg8,
            }
        )
    return in_maps, uniq_vals


def kernel(**inputs):
    in_maps, uniq_vals = _make_in_maps(inputs)
    nc = _get_nc()
    res = run_bass_kernel_spmd(nc, in_maps, list(range(NCORES))).results
    full = np.empty((NT, BS, V), dtype=np.float32)
    for c in range(NCORES):
        o = res[c]["out"].reshape(NT, NCHK, CHP)[:, :, 0:RCH].reshape(NT, V)
        full[:, c, :] = o.astype(np.float32)
        vals = uniq_vals[c]
        fix = res[c]["out_fix"][:, : len(vals)].astype(np.float32)
        full[:, c, vals] = fix
    return full


# revision 22
# speedup vs baseline: 1.0441x; 1.0441x over previous
"""CopyGenerator kernel for Trainium2 (Bass/Tile), batch-parallel over 8 cores.

Core c owns batch c end-to-end. Key trick vs the dense baseline: for every
vocab column v NOT hit by src, the output is an affine function of the raw
generation score,
    out_v = log((1-a)/Z * exp(s_v)) = s_v + ln((1-a)/Z) = s_v + lnc1,
so pass 2 is a single DVE add per block (no dense Ln / blend / one-hot
matmul / scatter). The <=128 distinct src columns are emitted separately as
    out_fix_u = ln(e_u + Z*cpn_u) + lnc1,   cpn = (a/(1-a))*cp,
a tiny [128 x 128] side output; the host overwrites those columns during
the unshard (it knows the indices).

Padded vocab layout: V' = 32768 = 64 chunks of 512, each holding 500 real
vocab columns + 12 zero-embedding pads. This makes every matmul a full
512-f32 PSUM bank, every DVE copy contiguous, and every emb/out DMA an
8KB-per-partition contiguous transfer (host pre-packs emb into per-block
layout and strips the pads from the output). Pads contribute exp(0)=1 to
the softmax sum, corrected exactly via Z -= 768.

PE p-state note: the Tensor engine only reaches 2.4 GHz after ~3us of
continuous execution; any stall drops it to 1.2 GHz. Hence: deep emb
prefetch (bufs=6), attention emitted early (after block 1), attention
biases folded into the DVE PSUM->SBUF copies instead of extra matmuls,
and x/copy-gate computed with one 512-col matmul + a DVE dot.

Device pipeline per core (vocab' in 2048-col blocks; first two half-width):
  pass 1: DMA emb block -> gen matmul (PE fp8 DoubleRowSwInterleave) ->
          sc = s (f16, DVE copy/32) + exp(s) on ACT into scratch with fused
          row-sum -> Z
  attention after block 1, then fixup payload (E, cp, sg, e_g, cpn)
  Z -> lnc1 -> out_fix (tiny) ; pass 2 blocks only wait on lnc1:
  pass 2: out_f16 = sc + lnc1 per 4096-col block (DVE) -> DMA out
Output f16, host strips pads, patches fix columns, upcasts to f32.
"""

import sys

sys.path.insert(0, "/opt/trn_rl_repo")

import numpy as np

from concourse import bass, bacc, mybir
import concourse.tile as tile
from concourse.bass_utils import run_bass_kernel_spmd

NT, NS, BS, D, V = 128, 128, 8, 512, 32000
NCORES = 8
P = 128
KC = D // P  # 4 contraction chunks of 128
G = KC // 2  # 2 DoubleRow pair-groups (256-deep each)
RCH = 500  # real vocab cols per 512-chunk
CHP = 512  # padded chunk = one full PSUM bank
NCHK = V // RCH  # 64 chunks
VP = NCHK * CHP  # 32768 padded vocab
NPAD = NCHK * (CHP - RCH)  # 768 pad columns, each contributing exp(0)=1
WCH = 2 * CHP  # 1024: cols per exp activate / sc copy (2 banks)
DCH = 2 * WCH  # 2048: cols per emb DMA block
NDMA = VP // DCH  # 16
NZ = VP // WCH  # 32 partial-Z columns
# fine-grained half blocks at start (smooth 3-queue startup) and end
# (short Z-critical tail); 2048-col blocks in the middle
BLOCKS = (
    [(k * WCH, WCH) for k in range(6)]
    + [(6 * WCH + k * DCH, DCH) for k in range(12)]
    + [(VP - 2 * WCH, WCH), (VP - WCH, WCH)]
)
assert sum(w for _, w in BLOCKS) == VP
NPQ = 128  # fixup payload columns (unique src values, -1 padded)
ESCALE = 32.0  # host scales emb by 32 into fp8e4 normal range; exp undoes
F32 = mybir.dt.float32
F16 = mybir.dt.float16
F8 = mybir.dt.float8e4
AF = mybir.ActivationFunctionType
ALU = mybir.AluOpType
DR = mybir.MatmulPerfMode.DoubleRowSwInterleave
INV_SQRT_D = 1.0 / float(np.sqrt(np.float32(D)))


def build_kernel():
    nc = bacc.Bacc(
        "TRN2",
        target_bir_lowering=False,
        debug=False,
        enable_asserts=False,
        num_devices=NCORES,
    )
    emb8b = nc.dram_tensor("emb8b", [P, 2 * G * VP], F8, kind="ExternalInput").ap()
    hh8 = nc.dram_tensor("hh8", [P, G, 2 * NT], F8, kind="ExternalInput").ap()
    hhT = nc.dram_tensor("hhT", [P, KC, 2, P], F16, kind="ExternalInput").ap()
    qwT = nc.dram_tensor("qwT", [P, KC, D], F16, kind="ExternalInput").ap()
    qbT = nc.dram_tensor("qbT", [P, KC], F32, kind="ExternalInput").ap()
    qbbc = nc.dram_tensor("qbbc", [P, D], F16, kind="ExternalInput").ap()
    w2bc = nc.dram_tensor("w2bc", [P, D], F32, kind="ExternalInput").ap()
    b2bc = nc.dram_tensor("b2bc", [P, 1], F32, kind="ExternalInput").ap()
    iden = nc.dram_tensor("iden", [P, P], F32, kind="ExternalInput").ap()
    src = nc.dram_tensor("src", [NS, 1], F32, kind="ExternalInput").ap()
    valr = nc.dram_tensor("valr", [P, NPQ], F32, kind="ExternalInput").ap()
    embg8 = nc.dram_tensor("embg8", [P, G, 2 * NPQ], F8, kind="ExternalInput").ap()
    out = nc.dram_tensor("out", [NT, VP], F16, kind="ExternalOutput").ap()
    out_fix = nc.dram_tensor("out_fix", [NT, NPQ], F16, kind="ExternalOutput").ap()

    with tile.TileContext(nc) as tc:
        _emit(
            nc, tc, emb8b, hh8, hhT, qwT, qbT, qbbc, w2bc, b2bc, iden, src, valr,
            embg8, out, out_fix,
        )
    nc.compile()
    return nc


def _emit(
    nc, tc, emb8b, hh8, hhT, qwT, qbT, qbbc, w2bc, b2bc, iden, src, valr,
    embg8, out, out_fix,
):
    with (
        tc.tile_pool(name="persist", bufs=1) as pw,
        tc.tile_pool(name="small", bufs=2) as psm,
        tc.tile_pool(name="scr", bufs=3) as pscr,
        tc.tile_pool(name="ps_m", bufs=2, space="PSUM") as ps_m,
        tc.tile_pool(name="ps_fix", bufs=1, space="PSUM") as ps_fix,
        tc.tile_pool(name="ps_gen", bufs=2, space="PSUM") as ps_gen,
    ):
        # ---- persistent SBUF ----
        sc_sb = pw.tile([P, VP], F16)  # (t, v') raw gen scores - 64KB/part
        hh_sb = pw.tile([P, KC, 2, P], F16)  # (d, kc, {tgt,src}, t/s)
        hh8_sb = pw.tile([P, G, 2 * NT], F8)  # (d, g, swi(t)) DR weights
        qwT_sb = pw.tile([P, KC, D], F16)  # (d, kc, i)
        qbT_sb = pw.tile([P, KC], F32)  # (i % P, ic) q bias column
        qbbc_sb = pw.tile([P, D], F16)  # q bias row, tiled over partitions
        w2bc_sb = pw.tile([P, D], F32)  # fused copy-gate weight row, tiled
        b2bc_sb = pw.tile([P, 1], F32)
        qkT_sb = pw.tile([P, KC, 2, P], F16)  # (i, ic, {q,k}, t/s)
        k_sb = pw.tile([P, D], F16)  # (s, i)
        xw_sb = pw.tile([P, D], F32)  # x * w2 elementwise
        attn_sb = pw.tile([P, NS], F32)  # (t, s)
        attnT_sb = pw.tile([P, NT], F16)  # (s, t)
        a_sb = pw.tile([P, 1], F32)  # (t,)
        src_sb = pw.tile([P, 1], F32)  # (s,)
        valr_sb = pw.tile([P, NPQ], F32)
        embg8_sb = pw.tile([P, G, 2 * NPQ], F8)
        E_sb = pw.tile([P, NPQ], F16)  # (s, q) src==valr selection
        eg_sb = pw.tile([P, NPQ], F32)  # (t, q) exp(sg)
        cpn_sb = pw.tile([P, NPQ], F32)  # (t, q) (a/(1-a)) * cp
        tmp_sb = pw.tile([P, NPQ], F32)
        lntmp_sb = pw.tile([P, NPQ], F32)
        fix_sb = pw.tile([P, NPQ], F16)
        identity = pw.tile([P, P], F32)
        zparts = pw.tile([P, NZ], F32)
        zcol = pw.tile([P, 1], F32)
        oma_sb = pw.tile([P, 1], F32)
        aroma_sb = pw.tile([P, 1], F32)
        lnc1_sb = pw.tile([P, 1], F32)

        # preload ACT function tables (Exp/Ln/Sigmoid) so the lazy
        # ACT_TABLE_LOADs don't land mid-stream or in the Z->lnc1 chain
        dum = psm.tile([1, 1], F32, tag="dum")
        dum2 = psm.tile([1, 1], F32, tag="dum2")
        nc.vector.memset(dum[:], 1.0)
        nc.scalar.activation(out=dum2[:], in_=dum[:], func=AF.Exp)
        nc.scalar.activation(out=dum2[:], in_=dum[:], func=AF.Ln)
        nc.scalar.activation(out=dum2[:], in_=dum[:], func=AF.Sigmoid)

        # gen matmul dep goes first on the sync queue so the PE can start
        # asap; attention deps ride the idle gpsimd queue; emb blocks 0-5
        # alternate sync/scalar, later ones round-robin all three
        nc.sync.dma_start(out=hh8_sb[:], in_=hh8)

        def emit_smalls_a():
            nc.gpsimd.dma_start(out=hh_sb[:], in_=hhT)
            nc.gpsimd.dma_start(out=qwT_sb[:], in_=qwT)
            nc.gpsimd.dma_start(out=qbT_sb[:], in_=qbT)
            nc.gpsimd.dma_start(out=qbbc_sb[:], in_=qbbc)
            nc.gpsimd.dma_start(out=identity[:], in_=iden)

        def emit_smalls_b():
            nc.gpsimd.dma_start(out=src_sb[:], in_=src)
            nc.gpsimd.dma_start(out=valr_sb[:], in_=valr)
            nc.gpsimd.dma_start(out=embg8_sb[:], in_=embg8)
            nc.gpsimd.dma_start(out=w2bc_sb[:], in_=w2bc)
            nc.gpsimd.dma_start(out=b2bc_sb[:], in_=b2bc)

        def emit_attention():
            # q,k = h @ q_w.T + qb for {htgt, hsrc}; bias folded into the
            # PSUM->SBUF copy as a per-partition (i) scalar add
            for ic in range(KC):
                qkT_ps = ps_m.tile([P, 2 * P], F32, tag="m")
                for kc in range(KC):
                    nc.tensor.matmul(
                        out=qkT_ps[:],
                        lhsT=qwT_sb[:, kc, ic * P : (ic + 1) * P],
                        rhs=hh_sb[:, kc],
                        start=(kc == 0),
                        stop=(kc == KC - 1),
                    )
                nc.vector.tensor_scalar(
                    out=qkT_sb[:, ic],
                    in0=qkT_ps[:].rearrange("i (w t) -> i w t", t=P),
                    scalar1=qbT_sb[:, ic : ic + 1],
                    scalar2=None,
                    op0=ALU.add,
                )

            # k in (s, i) layout for the x matmul; bias is a row -> DVE add
            k_ps = ps_m.tile([P, D], F32, tag="m")
            for kc in range(KC):
                nc.tensor.matmul(
                    out=k_ps[:],
                    lhsT=hh_sb[:, kc, 1, :],
                    rhs=qwT_sb[:, kc, :],
                    start=(kc == 0),
                    stop=(kc == KC - 1),
                )
            nc.vector.tensor_tensor(
                out=k_sb[:], in0=k_ps[:], in1=qbbc_sb[:], op=ALU.add
            )

            s_ps = ps_m.tile([P, P], F32, tag="m")
            for ic in range(KC):
                nc.tensor.matmul(
                    out=s_ps[:],
                    lhsT=qkT_sb[:, ic, 0, :],
                    rhs=qkT_sb[:, ic, 1, :],
                    start=(ic == 0),
                    stop=(ic == KC - 1),
                )
            m_col = psm.tile([P, 1], F32, tag="m")
            negm = psm.tile([P, 1], F32, tag="negm")
            zatt = psm.tile([P, 1], F32, tag="zatt")
            rz = psm.tile([P, 1], F32, tag="rz")
            nc.vector.reduce_max(
                out=m_col[:], in_=s_ps[:], axis=mybir.AxisListType.X
            )
            nc.vector.tensor_scalar_mul(negm[:], m_col[:], -INV_SQRT_D)
            nc.scalar.activation(
                out=attn_sb[:],
                in_=s_ps[:],
                func=AF.Exp,
                bias=negm[:],
                scale=INV_SQRT_D,
                accum_out=zatt[:],
            )
            nc.vector.reciprocal(rz[:], zatt[:])
            nc.vector.tensor_scalar_mul(attn_sb[:], attn_sb[:], rz[:])

            t_ps = ps_m.tile([P, P], F32, tag="m")
            nc.tensor.transpose(t_ps[:], attn_sb[:], identity[:])
            nc.vector.tensor_copy(out=attnT_sb[:], in_=t_ps[:])

            # x = attn @ v (v == k): one 512-col matmul into (t, i) layout;
            # copy gate a = sigmoid(x . w2 + b2) via DVE dot
            x_ps = ps_m.tile([P, D], F32, tag="m")
            nc.tensor.matmul(
                out=x_ps[:],
                lhsT=attnT_sb[:],
                rhs=k_sb[:],
                start=True,
                stop=True,
            )
            c_col = psm.tile([P, 1], F32, tag="ccol")
            nc.vector.tensor_tensor(
                out=xw_sb[:], in0=x_ps[:], in1=w2bc_sb[:], op=ALU.mult
            )
            nc.vector.reduce_sum(
                out=c_col[:], in_=xw_sb[:], axis=mybir.AxisListType.X
            )
            nc.scalar.activation(
                out=a_sb[:], in_=c_col[:], func=AF.Sigmoid, bias=b2bc_sb[:]
            )

        def emit_fixup_payload():
            # oma = 1-a, aroma = a/(1-a)  (only needs a; Z comes later)
            roma = psm.tile([P, 1], F32, tag="roma")
            nc.vector.tensor_scalar(
                out=oma_sb[:],
                in0=a_sb[:],
                scalar1=-1.0,
                scalar2=1.0,
                op0=ALU.mult,
                op1=ALU.add,
            )
            nc.vector.reciprocal(roma[:], oma_sb[:])
            nc.vector.tensor_tensor(
                out=aroma_sb[:], in0=a_sb[:], in1=roma[:], op=ALU.mult
            )
            # E[s, q] = (src[s] == valr[q]); dead cols (valr=-1) never match
            nc.vector.tensor_scalar(
                out=E_sb[:],
                in0=valr_sb[:],
                scalar1=src_sb[:],
                scalar2=None,
                op0=ALU.is_equal,
            )
            # cp[t, q] = sum_s attn[t, s] * E[s, q];  cpn = aroma * cp
            cpu_ps = ps_fix.tile([P, CHP], F32, tag="cpu")
            nc.tensor.matmul(
                out=cpu_ps[:, 0:NPQ],
                lhsT=attnT_sb[:],
                rhs=E_sb[:],
                start=True,
                stop=True,
            )
            nc.vector.tensor_scalar(
                out=cpn_sb[:],
                in0=cpu_ps[:, 0:NPQ],
                scalar1=aroma_sb[:],
                scalar2=None,
                op0=ALU.mult,
            )
            # sg[t, q] = ESCALE * (htgt[t] . emb[valr[q]])  (same fp8 inputs
            # as the gen matmul -> identical rounding); e_g = exp(sg/ESCALE)
            sg_ps = ps_fix.tile([P, CHP], F32, tag="sg")
            for g in range(G):
                nc.tensor.matmul(
                    out=sg_ps[:, 0:NPQ],
                    lhsT=hh8_sb[:, g],
                    rhs=embg8_sb[:, g].rearrange("p (i q) -> p i q", i=2),
                    perf_mode=DR,
                    start=(g == 0),
                    stop=(g == G - 1),
                )
            nc.scalar.activation(
                out=eg_sb[:], in_=sg_ps[:, 0:NPQ], func=AF.Exp, scale=1.0 / ESCALE
            )

        # ---- pass 1: s = htgt @ embT, fp8 DoubleRow, streamed ----
        # first two loads are half-width so the PE starts sooner
        blocks = BLOCKS
        engs3 = [nc.sync, nc.scalar, nc.gpsimd]
        off = 0
        with (
            tc.tile_pool(name="embh", bufs=4) as pembh,
            tc.tile_pool(name="embst", bufs=8) as pemb,
        ):
            for bi, (v0, width) in enumerate(blocks):
                if width < DCH:
                    emb_t = pembh.tile([P, G, 2, width], F8, tag="embh")
                else:
                    emb_t = pemb.tile([P, G, 2, width], F8, tag="emb")
                eng = engs3[bi % 3]
                eng.dma_start(
                    out=emb_t[:].rearrange("p g i v -> p (g i v)"),
                    in_=emb8b[:, off : off + 2 * G * width],
                )
                off += 2 * G * width
                if bi == 2:
                    emit_smalls_a()
                    emit_smalls_b()
                for w in range(width // WCH):
                    # g-major: the DoubleRow weight set stays loaded for two
                    # consecutive matmuls (both PSUM banks of this group)
                    g_ps = ps_gen.tile([P, 2, CHP], F32, tag="g")
                    for g in range(G):
                        for h in range(2):
                            c0 = w * WCH + h * CHP
                            nc.tensor.matmul(
                                out=g_ps[:, h, :],
                                lhsT=hh8_sb[:, g],
                                rhs=emb_t[:, g, :, c0 : c0 + CHP],
                                perf_mode=DR,
                                start=(g == 0),
                                stop=(g == G - 1),
                            )
                    nz = (v0 + w * WCH) // WCH
                    scr = pscr.tile([P, 2, CHP], F16, tag="scr")
                    nc.scalar.activation(
                        out=scr[:],
                        in_=g_ps[:],
                        func=AF.Exp,
                        scale=1.0 / ESCALE,
                        accum_out=zparts[:, nz : nz + 1],
                    )
                    nc.vector.tensor_scalar(
                        out=sc_sb[:, nz * WCH : (nz + 1) * WCH].rearrange(
                            "t (h c) -> t h c", c=CHP
                        ),
                        in0=g_ps[:],
                        scalar1=1.0 / ESCALE,
                        scalar2=None,
                        op0=ALU.mult,
                    )
                if bi == 6:
                    emit_attention()
                    emit_fixup_payload()

        # ---- Z (pad-corrected), lnc1 = ln((1-a)/Z) ----
        zraw = psm.tile([P, 1], F32, tag="zraw")
        rzg = psm.tile([P, 1], F32, tag="rzg")
        c1t = psm.tile([P, 1], F32, tag="c1t")
        nc.vector.reduce_sum(out=zraw[:], in_=zparts[:], axis=mybir.AxisListType.X)
        nc.vector.tensor_scalar(
            out=zcol[:], in0=zraw[:], scalar1=-float(NPAD), scalar2=None, op0=ALU.add
        )
        nc.vector.reciprocal(rzg[:], zcol[:])
        nc.vector.tensor_tensor(out=c1t[:], in0=oma_sb[:], in1=rzg[:], op=ALU.mult)
        nc.scalar.activation(out=lnc1_sb[:], in_=c1t[:], func=AF.Ln)

        def emit_fix_chain():
            # out_fix = ln(e_g + Z*cpn) + lnc1 (off the pass-2 critical path)
            nc.vector.scalar_tensor_tensor(
                out=tmp_sb[:],
                in0=cpn_sb[:],
                scalar=zcol[:],
                in1=eg_sb[:],
                op0=ALU.mult,
                op1=ALU.add,
            )
            nc.scalar.activation(out=lntmp_sb[:], in_=tmp_sb[:], func=AF.Ln)
            nc.vector.tensor_scalar(
                out=fix_sb[:],
                in0=lntmp_sb[:],
                scalar1=lnc1_sb[:],
                scalar2=None,
                op0=ALU.add,
            )
            nc.gpsimd.dma_start(out=out_fix, in_=fix_sb[:])

        # ---- pass 2: out = sc + lnc1, f16, stream out ----
        OCH = 2 * DCH
        oblocks = (
            [(0, DCH)]
            + [(DCH + no * OCH, OCH) for no in range(7)]
            + [(VP - DCH, DCH)]
        )
        with tc.tile_pool(name="p2", bufs=3) as p2:
            for oi, (o0, owidth) in enumerate(oblocks):
                outt = p2.tile([P, OCH], F16, tag="outt")
                nc.vector.tensor_scalar(
                    out=outt[:, 0:owidth],
                    in0=sc_sb[:, o0 : o0 + owidth],
                    scalar1=lnc1_sb[:],
                    scalar2=None,
                    op0=ALU.add,
                )
                engs3[oi % 3].dma_start(
                    out=out[:, o0 : o0 + owidth], in_=outt[:, 0:owidth]
                )
                if oi == 1:
                    emit_fix_chain()


_NC_CACHE = []


def _get_nc():
    if not _NC_CACHE:
        _NC_CACHE.append(build_kernel())
    return _NC_CACHE[0]


def _f8(x):
    return x.astype(mybir.dt.np(F8))


def _make_in_maps(inputs):
    htgt = np.asarray(inputs["htgt"], dtype=np.float32)
    hsrc = np.asarray(inputs["hsrc"], dtype=np.float32)
    src = np.asarray(inputs["src"])
    srcf = src.astype(np.float32)  # exact for v < 2^24
    emb = np.asarray(inputs["emb_weight"], dtype=np.float32)
    q_w = np.asarray(inputs["q_w"], dtype=np.float32)
    q_b = np.asarray(inputs["q_b"], dtype=np.float32)
    f_w = np.asarray(inputs["f_w"], dtype=np.float32)
    f_b = np.asarray(inputs["f_b"], dtype=np.float32)
    copy_w = np.asarray(inputs["copy_w"], dtype=np.float32)
    copy_b = np.asarray(inputs["copy_b"], dtype=np.float32)

    # (G, 2, P, V): d = g*256 + i*128 + p, scaled into fp8e4 normal range
    emb8 = np.ascontiguousarray(_f8((emb.T * ESCALE).reshape(G, 2, P, V)))
    # padded layout: 64 chunks of 512 = 500 real + 12 zero cols
    embp = np.zeros((G, 2, P, NCHK, CHP), dtype=emb8.dtype)
    embp[:, :, :, :, 0:RCH] = emb8.reshape(G, 2, P, NCHK, RCH)
    embp = embp.reshape(G, 2, P, VP)
    # pack per (block, g, i, v') so each block is one contiguous
    # 8KB-per-partition DMA read
    segs = [
        np.transpose(embp[:, :, :, v0 : v0 + w], (2, 0, 1, 3)).reshape(P, -1)
        for (v0, w) in BLOCKS
    ]
    emb8b = np.ascontiguousarray(np.concatenate(segs, axis=1))

    qwT = np.ascontiguousarray(
        np.transpose(q_w.T.astype(np.float16).reshape(KC, P, D), (1, 0, 2))
    )
    qb16 = q_b.astype(np.float16)
    qbT = np.ascontiguousarray(q_b.astype(np.float32).reshape(KC, P).T)
    qbbc = np.ascontiguousarray(np.tile(qb16.reshape(1, D), (P, 1)))
    w2c = (f_w.T @ copy_w[0]).astype(np.float32)
    w2bc = np.ascontiguousarray(np.tile(w2c.reshape(1, D), (P, 1)))
    b2v = np.float32(copy_w[0] @ f_b + copy_b[0])
    b2bc = np.full((P, 1), b2v, np.float32)
    iden = np.ascontiguousarray(np.eye(P, dtype=np.float32))

    in_maps = []
    uniq_vals = []
    for c in range(NCORES):
        hh = np.stack([htgt[:, c, :].T, hsrc[:, c, :].T], axis=1)  # (D, 2, P)
        hhT = np.ascontiguousarray(
            np.transpose(hh.astype(np.float16).reshape(KC, P, 2, P), (1, 0, 2, 3))
        )
        # SwInterleave weight layout: per (g, partition): A/B pairs
        # interleaved per column, columns reversed
        h8 = _f8(htgt[:, c, :].T.reshape(G, 2, P, NT))
        rev = h8[:, :, :, ::-1]
        swi = np.empty((P, G, 2 * NT), dtype=h8.dtype)
        swi[:, :, 0::2] = np.transpose(rev[:, 0], (1, 0, 2))
        swi[:, :, 1::2] = np.transpose(rev[:, 1], (1, 0, 2))
        hh8 = np.ascontiguousarray(swi)

        # fixup prep: unique vocab values hit by this batch's src
        vals = np.unique(src[:, c].astype(np.int64))  # sorted, <= 128
        nu = len(vals)
        uniq_vals.append(vals)
        vq = np.full(NPQ, -1, np.int64)
        vq[:nu] = vals
        valr = np.ascontiguousarray(
            np.tile(vq.astype(np.float32).reshape(1, NPQ), (P, 1))
        )
        eg = np.zeros((G, 2, P, NPQ), dtype=emb8.dtype)
        eg[:, :, :, :nu] = emb8[:, :, :, vals]
        embg8 = np.ascontiguousarray(
            np.transpose(eg, (2, 0, 1, 3)).reshape(P, G, 2 * NPQ)
        )

        in_maps.append(
            {
                "emb8b": emb8b,
                "hh8": hh8,
                "hhT": hhT,
                "qwT": qwT,
                "qbT": qbT,
                "qbbc": qbbc,
                "w2bc": w2bc,
                "b2bc": b2bc,
                "iden": iden,
                "src": np.ascontiguousarray(srcf[:, c].reshape(NS, 1)),
                "valr": valr,
                "embg8": embg8,
            }
        )
    return in_maps, uniq_vals


def kernel(**inputs):
    in_maps, uniq_vals = _make_in_maps(inputs)
    nc = _get_nc()
    res = run_bass_kernel_spmd(nc, in_maps, list(range(NCORES))).results
    full = np.empty((NT, BS, V), dtype=np.float32)
    for c in range(NCORES):
        o = res[c]["out"].reshape(NT, NCHK, CHP)[:, :, 0:RCH].reshape(NT, V)
        full[:, c, :] = o.astype(np.float32)
        vals = uniq_vals[c]
        fix = res[c]["out_fix"][:, : len(vals)].astype(np.float32)
        full[:, c, vals] = fix
    return full
